# revision 5
# baseline (speedup 1.0000x reference)
"""Fused multi-head attention (B=2, N=2048, C=1024, H=16) on 8 TRN2 NeuronCores.

Sharding: core = (b, g) with b = batch (2) and g = head-group of 4 heads (4).
Each core computes, for its batch and 4 heads:
    qkv slice -> per-head softmax attention -> out-proj partial (row-parallel).
Host sums the 4 per-head-group proj partials per batch and adds b_proj.

Device algorithm (per core), matmuls in bf16:
  phase 1: qkT = (x @ Wqk)^T   [q/k feats on partitions, 2048 tokens]
           v   = x @ Wv        [2048 tokens, 4*64] (+ ones column per head)
  phase 2 (quad-ST): per (head pair hp, 512-row chunk rc), 16 key chunks kc:
           S^T for BOTH heads of the pair via 4 concurrent 64x64-quadrant
             matmuls (tile_position=(64*hh, 64*cc)): quadrant (hh, cc)
             contracts head hh's 64 features against kT2[feat half hh,
             keys kb+64cc : +64], writing st[64cc:+64, 512hh:+512].  All 4
             run concurrently on disjoint array quadrants (HW-verified
             ~3ns start stagger), so 256 keys x 512 tokens of S^T cost
             ~512 array cycles instead of 1024 -- the K=64 contraction no
             longer wastes half the array.
           exp: ONE [128,1024] ScalarE ACT covers both heads' tiles.
           PV: 2 matmuls (M=65: ones-column = softmax denominator row)
             accumulate outT^T / denominator over the 16 kc.
  phase 3: partial = out^T-matmul Wp -> bf16 -> DMA out

Schedule: ONE global software pipeline over 64 super-steps (2 kc each,
8 per block).  Per super-step the PE program is
  [wave(2k), wave(2k+1)] [PV pairs (2k-4, 2k-3)] [fill pops]
with the two ScalarE ACTs between.  Batching two waves back-to-back and
two PV pairs back-to-back hides the PE pipe-drain + LDWEIGHTS cost at each
array-config switch; the lag-4 PV emission keeps waves (the ACT feeders)
from ever queueing behind PV matmuls that wait on the previous ACT.
ScalarE is the binding engine (~1.0us/ACT); qkv+proj matmuls ride the fill
queue in the PE slack, paced per super-step, with proj(rc=2) deliberately
saved for the tail block where the PE otherwise starves waiting for the
final ACT backlog.  The DMA-paced prologue interleaves the qk00/qk20 (and
v0/v1) contractions chunk-by-chunk with dependency-free bf16 filler
matmuls so the PE HAM clock gate opens (~3.4us busy window) during the
input DMA instead of after it.
Note: the box drifts between "fast" and ~20% slower power states on minute
timescales; compare variants only via interleaved runs in one process.
"""

import os
from contextlib import ExitStack

import numpy as np

import concourse.bass as bass
import concourse.mybir as mybir
import concourse.tile as tile
from concourse import bacc
from concourse.bass_utils import run_bass_kernel_spmd

B, N, C = 2, 2048, 1024
HC = 4  # heads per core
D = 64
NCORES = 8
KC = C // 128  # 8 contraction chunks for phase 1
SCALE = D**-0.5  # 0.125

# "f32r" (fp32 data, full-rate PE mode), "bf16", or "f32" (4x slower PE)
MM_DT = os.environ.get("ATTN_MM_DT", "bf16")


def _np_in_dtype():
    if MM_DT == "bf16":
        import ml_dtypes

        return np.dtype(ml_dtypes.bfloat16)
    return np.dtype(np.float32)


def _prep(a):
    """Cast to the device input dtype; for f32r, pre-round to TF32 (RTNE)."""
    a = np.ascontiguousarray(a)
    if MM_DT != "f32r":
        return a.astype(_np_in_dtype())
    u = a.astype(np.float32).view(np.uint32)
    u = (u + 0x0FFF + ((u >> 13) & 1)) & np.uint32(0xFFFFE000)
    return u.view(np.float32)


def build_nc():
    f32 = mybir.dt.float32
    in_dt = {
        "bf16": mybir.dt.bfloat16,
        "f32r": mybir.dt.float32r,
        "f32": mybir.dt.float32,
    }[MM_DT]
    mm = lambda ap: ap  # noqa: E731

    out_dt = mybir.dt.bfloat16 if MM_DT == "bf16" else f32

    nc = bacc.Bacc("TRN2", target_bir_lowering=False, debug=False, num_devices=NCORES)
    xT_d = nc.dram_tensor("xT", [C, N], in_dt, kind="ExternalInput").ap()
    wqk_d = nc.dram_tensor("wqk", [C, 2 * HC * D], in_dt, kind="ExternalInput").ap()
    wv_d = nc.dram_tensor("wv", [C, HC * D], in_dt, kind="ExternalInput").ap()
    wp_d = nc.dram_tensor("wp", [HC * D, C], in_dt, kind="ExternalInput").ap()
    # bf16 proj partials: halves the output DMA (the tail's critical path);
    # the host accumulates the 4 partials per batch in f32.
    out_d = nc.dram_tensor("out", [N, C], out_dt, kind="ExternalOutput").ap()

    with tile.TileContext(nc) as tc:
        with (
            tc.tile_pool(name="const", bufs=1) as const,
            tc.tile_pool(name="ex", bufs=8) as expool,
            tc.tile_pool(name="den", bufs=6) as dpool,
            tc.tile_pool(name="stage", bufs=4) as stage,
            tc.tile_pool(name="stps", bufs=2, space="PSUM") as stps,
            tc.tile_pool(name="pvps", bufs=4, space="PSUM") as pvps,
        ):
            # persistent tiles
            # qkT chunks: 0 = q heads 0,1; 1 = q heads 2,3
            #   (head even -> partitions 0:64, odd -> 64:128)
            # kT2: same layout for k -- consumed in 64x64 slices by the
            #   quadrant ST matmuls, so no zero padding is needed.
            qkT_sb = const.tile([128, 2, N], in_dt, tag="qkT")
            kT2_sb = const.tile([128, 2, N], in_dt, tag="kT2")
            v_sb = const.tile([128, 16, HC, D + 1], in_dt, tag="v")
            wp_sb = const.tile([128, 2, C], in_dt, tag="wp")
            outT_sb = const.tile([128, 2, N], in_dt, tag="outT")
            xT_sb = const.tile([128, KC, N], in_dt, tag="xT")
            wqk_sb = const.tile([128, KC, 2 * HC * D], in_dt, tag="wqk")
            wv_sb = const.tile([128, KC, HC * D], in_dt, tag="wv")

            # ---- DMAs: the prologue-critical wqk / xT(nt=0) / wv chunks go
            # one-per-queue (Sync / GpSimd / Scalar HW DMA queues) so all
            # three stream in parallel and the interleaved qk00/qk20/v0/v1
            # prologue contractions can follow the data chunk-by-chunk.
            for kc in range(KC):
                nc.sync.dma_start(
                    wqk_sb[:, kc, :], wqk_d[kc * 128 : (kc + 1) * 128, :]
                )
                nc.gpsimd.dma_start(
                    xT_sb[:, kc, 0:512], xT_d[kc * 128 : (kc + 1) * 128, 0:512]
                )
                nc.scalar.dma_start(
                    wv_sb[:, kc, :], wv_d[kc * 128 : (kc + 1) * 128, :]
                )

            def dma_xt(nt, q):
                for kc in range(KC):
                    q.dma_start(
                        xT_sb[:, kc, nt * 512 : (nt + 1) * 512],
                        xT_d[kc * 128 : (kc + 1) * 128, nt * 512 : (nt + 1) * 512],
                    )

            dma_xt(1, nc.sync)
            dma_xt(2, nc.gpsimd)
            dma_xt(3, nc.sync)
            for c2 in range(2):
                nc.scalar.dma_start(wp_sb[:, c2, :], wp_d[c2 * 128 : (c2 + 1) * 128, :])

            # ---- one-time fills (run during the DMA wait) ----
            zbf = const.tile([64, 512], in_dt, tag="zbf")
            nc.vector.memset(zbf[:], 0.0)
            zsrc = const.tile([64, 512], f32, tag="zsrc")
            nc.vector.memset(zsrc[:], 0.0)
            ones_f32 = const.tile([128, 16, HC, 1], f32, tag="ones")
            nc.vector.memset(ones_f32[:], 1.0)
            nc.vector.tensor_copy(v_sb[:, :, :, D : D + 1], ones_f32[:])

            # HAM warmup / DMA-gap fillers: dependency-free bf16 matmuls on
            # the zero tile (never consumed).  Woven between the DMA-gated
            # prologue matmuls they keep the PE busy through the input
            # stream, so the HAM clock gate opens at ~4us instead of ~20us.
            wps = stps.tile([128, 1024], f32, tag="st", name="wps")

            def filler(n=256):
                nc.tensor.matmul(
                    wps[:, 0:n], zbf[:, 0:128], zbf[:, 0:n], start=True, stop=True
                )

            # ---- emission helpers ----
            def qk_chunk(mf, nt):
                """One psum of (x @ Wqk)^T: feat chunk mf, token chunk nt.
                wqk feat chunks: 0 = q heads 0,1; 1 = q heads 2,3;
                2 = k heads 0,1; 3 = k heads 2,3."""
                ps = pvps.tile([128, 512], f32, tag="pv", name="pv")
                for kc in range(KC):
                    nc.tensor.matmul(
                        ps,
                        mm(wqk_sb[:, kc, mf * 128 : (mf + 1) * 128]),
                        mm(xT_sb[:, kc, nt * 512 : (nt + 1) * 512]),
                        start=(kc == 0),
                        stop=(kc == KC - 1),
                    )
                nts = slice(nt * 512, (nt + 1) * 512)
                if mf < 2:
                    nc.vector.tensor_copy(qkT_sb[:, mf, nts], ps)
                else:
                    nc.vector.tensor_copy(kT2_sb[:, mf - 2, nts], ps)

            def v_chunk(t):
                """One psum of v = x @ Wv for token(=key) chunk t, all heads."""
                ps = pvps.tile([128, 512], f32, tag="pv", name="pv")[:, : HC * D]
                for kc in range(KC):
                    nc.tensor.matmul(
                        ps,
                        mm(xT_sb[:, kc, t * 128 : (t + 1) * 128]),
                        mm(wv_sb[:, kc, :]),
                        start=(kc == 0),
                        stop=(kc == KC - 1),
                    )
                nc.vector.tensor_copy(
                    v_sb[:, t, :, 0:D], ps.rearrange("p (h d) -> p h d", h=HC)
                )

            sg2_of = {}

            def proj_chunk(t, nf):
                """partial[t*128:(t+1)*128, nf*512:(nf+1)*512] = out @ Wp.
                Both nf halves stage into one [128,1024] tile; the DMA (2KB
                rows, half the packets) fires once per token chunk."""
                ps = pvps.tile([128, 512], f32, tag="pv", name="pv")
                for c2 in range(2):
                    nc.tensor.matmul(
                        ps,
                        mm(outT_sb[:, c2, t * 128 : (t + 1) * 128]),
                        mm(wp_sb[:, c2, nf * 512 : (nf + 1) * 512]),
                        start=(c2 == 0),
                        stop=(c2 == 1),
                    )
                if nf == 0:
                    sg2_of[t] = stage.tile(
                        [128, 1024], out_dt, tag="sg2", name="sg2", bufs=2
                    )
                sg = sg2_of[t]
                nc.vector.tensor_copy(sg[:, nf * 512 : (nf + 1) * 512], ps)
                if nf == 1:
                    nc.sync.dma_start(out_d[t * 128 : (t + 1) * 128, :], sg)
                    del sg2_of[t]

            def proj_tail(t):
                """Both nf halves of token chunk t in one stps-pool psum
                (free after the last exp): fewer, wider tail ops + 2KB-row
                output DMA."""
                ps = stps.tile([128, 1024], f32, tag="st", name="st")
                for nf in range(2):
                    for c2 in range(2):
                        nc.tensor.matmul(
                            ps[:, nf * 512 : (nf + 1) * 512],
                            mm(outT_sb[:, c2, t * 128 : (t + 1) * 128]),
                            mm(wp_sb[:, c2, nf * 512 : (nf + 1) * 512]),
                            start=(c2 == 0),
                            stop=(c2 == 1),
                        )
                sg = stage.tile([128, 1024], out_dt, tag="sg2", name="sg2", bufs=2)
                nc.vector.tensor_copy(sg, ps)
                # tail runs after the last ACT, so the scalar queue is free
                (nc.sync if t % 2 == 0 else nc.scalar).dma_start(
                    out_d[t * 128 : (t + 1) * 128, :], sg
                )

            # fill queue: work interleaved into the pipeline's PE slack
            fills = []

            def queue_proj(rc):
                fills.extend(
                    [
                        lambda t=t, nf=nf: proj_chunk(t, nf)
                        for t in range(4 * rc, 4 * rc + 4)
                        for nf in range(2)
                    ]
                )

            # ---- prologue: exactly what super-step 0 needs (q heads01 rc0,
            # kT2 pair0 keys 0:512, v keys 0:256), contracted chunk-by-chunk
            # as the DMA delivers, with fillers bridging the gaps.  qk00/qk20
            # (and v0/v1) share input chunks, so they interleave kc-wise and
            # each DMA arrival feeds two back-to-back matmuls.
            for _ in range(4):
                filler()
            psA = pvps.tile([128, 512], f32, tag="pv", name="psA")
            psB = pvps.tile([128, 512], f32, tag="pv", name="psB")
            for kc in range(KC):
                filler()
                for ps, mf in ((psA, 0), (psB, 2)):
                    nc.tensor.matmul(
                        ps,
                        mm(wqk_sb[:, kc, mf * 128 : (mf + 1) * 128]),
                        mm(xT_sb[:, kc, 0:512]),
                        start=(kc == 0),
                        stop=(kc == KC - 1),
                    )
            nc.vector.tensor_copy(qkT_sb[:, 0, 0:512], psA)
            nc.vector.tensor_copy(kT2_sb[:, 0, 0:512], psB)
            psC = pvps.tile([128, 512], f32, tag="pv", name="psC")
            psD = pvps.tile([128, 512], f32, tag="pv", name="psD")
            for kc in range(KC):
                filler(128)
                for ps, t in ((psC, 0), (psD, 1)):
                    nc.tensor.matmul(
                        ps[:, : HC * D],
                        mm(xT_sb[:, kc, t * 128 : (t + 1) * 128]),
                        mm(wv_sb[:, kc, :]),
                        start=(kc == 0),
                        stop=(kc == KC - 1),
                    )
            for ps, t in ((psC, 0), (psD, 1)):
                nc.vector.tensor_copy(
                    v_sb[:, t, :, 0:D],
                    ps[:, : HC * D].rearrange("p (h d) -> p h d", h=HC),
                )

            # ---- attention: ONE global software pipeline over 64 supers ----
            border = [
                (0, 0),
                (0, 1),
                (1, 0),
                (0, 2),
                (1, 1),
                (0, 3),
                (1, 2),
                (1, 3),
            ]
            # fill order + per-super pop counts (see module docstring):
            # block0: v2..v15 + kT2 chunks qk2x + q chunk qk01, paced to
            # land >=1 super before their consumer; block1: pair-1 q/kT2
            # chunks (urgent: block2 = (1,0)) then the rest; proj(0..1)
            # spread mid-run; proj(2) saved for the ACT-backlog tail.
            fills.extend(
                [
                    lambda: v_chunk(2),
                    lambda: qk_chunk(2, 1),
                    lambda: v_chunk(3),
                    lambda: v_chunk(4),
                    lambda: v_chunk(5),
                    lambda: qk_chunk(2, 2),
                    lambda: v_chunk(6),
                    lambda: v_chunk(7),
                    lambda: v_chunk(8),
                    lambda: v_chunk(9),
                    lambda: qk_chunk(2, 3),
                    lambda: v_chunk(10),
                    lambda: v_chunk(11),
                    lambda: v_chunk(12),
                    lambda: v_chunk(13),
                    lambda: qk_chunk(0, 1),
                    lambda: v_chunk(14),
                    lambda: v_chunk(15),
                ]
            )
            fills2 = [
                lambda: qk_chunk(1, 0),
                lambda: qk_chunk(3, 0),
                lambda: qk_chunk(3, 1),
                lambda: qk_chunk(3, 2),
                lambda: qk_chunk(3, 3),
                lambda: qk_chunk(0, 2),
                lambda: qk_chunk(1, 1),
                lambda: qk_chunk(0, 3),
                lambda: qk_chunk(1, 2),
                lambda: qk_chunk(1, 3),
            ]
            sched = (
                [0, 2, 2, 2, 2, 3, 2, 3]  # block0: 16 of the 18 v/k fills
                + [2, 1, 1, 1, 1, 1, 0, 1]  # block1: v14,v15 + 6 of fills2
                + [0, 1, 0, 1, 0, 1, 0, 1]  # block2: rest of fills2
                + [0, 0, 1, 1, 1, 1, 1, 1]  # block3: proj0 (queued s25)
                + [1, 1, 0, 0, 0, 0, 0, 0]  # block4
                + [0, 0, 1, 1, 1, 1, 1, 1]  # block5: proj1 (queued s41)
                + [1, 1, 0, 0, 0, 0, 0, 0]  # block6
                + [0, 1, 1, 1, 1, 1, 1, 1]  # block7: proj2 -> PE-starved tail
            )
            bst = [None] * 8  # per-block pipeline state
            exs = {}

            def wave(s):
                bi, kc = s // 16, s % 16
                if kc == 0:
                    hp, rc = border[bi]
                    rcs = slice(rc * 512, (rc + 1) * 512)
                    bst[bi] = {
                        "heads": (2 * hp, 2 * hp + 1),
                        "pv": {
                            h: pvps.tile([128, 512], f32, tag="pv", name="pv")
                            for h in (2 * hp, 2 * hp + 1)
                        },
                        "q": (qkT_sb[0:64, hp, rcs], qkT_sb[64:128, hp, rcs]),
                    }
                    if bi == 1:
                        fills.extend(fills2)
                hp, rc = border[bi]
                st8 = bst[bi]
                st = stps.tile([128, 1024], f32, tag="st", name="st")
                kb = kc * 128
                for hh in range(2):  # head within pair -> array row half
                    for cc in range(2):  # key half -> array col half
                        nc.tensor.matmul(
                            st[64 * cc : 64 * cc + 64, 512 * hh : 512 * hh + 512],
                            mm(
                                kT2_sb[
                                    64 * hh : 64 * hh + 64,
                                    hp,
                                    kb + 64 * cc : kb + 64 * cc + 64,
                                ]
                            ),
                            mm(st8["q"][hh]),
                            start=True,
                            stop=True,
                            tile_position=(64 * hh, 64 * cc),
                        )
                return st

            def act(s, st):
                ex = expool.tile([128, 1024], in_dt, tag="ex", name="ex")
                nc.scalar.activation(
                    ex, st, mybir.ActivationFunctionType.Exp, scale=SCALE
                )
                exs[s] = ex

            def pv_pair(s):
                bi, kc = s // 16, s % 16
                st8 = bst[bi]
                ex = exs.pop(s)
                for hh, h in enumerate(st8["heads"]):
                    nc.tensor.matmul(
                        st8["pv"][h][: D + 1, :],
                        mm(v_sb[:, kc, h, :]),
                        mm(ex[:, 512 * hh : 512 * hh + 512]),
                        start=(kc == 0),
                        stop=(kc == 15),
                    )
                if kc == 15:
                    block_end(bi)

            def block_end(bi):
                """Denominator chain + normalize (+ tail proj) for block bi."""
                hp, rc = border[bi]
                st8 = bst[bi]
                heads, pv = st8["heads"], st8["pv"]
                tail = bi == 7
                if tail:
                    # the tail denominator chain leaves the PE idle just over
                    # the HAM re-throttle window; dependency-free filler
                    # matmuls (f32 on zsrc, never consumed) bridge it.
                    wmt = stps.tile([128, 1024], f32, tag="st", name="wmt")
                    for _ in range(8):
                        nc.tensor.matmul(
                            wmt[:, 0:256],
                            zsrc[:, 0:128],
                            zsrc[:, 0:256],
                            start=True,
                            stop=True,
                        )
                # both recips first: DVE stays busy while GpSimd runs the
                # first broadcast.  approx-fast recip: ~51 ULP, ~5x faster
                # than the iterative divide -- the denominator only needs
                # ~1e-2 relative.
                dens, rbcs = {}, {}
                for h in heads:
                    dens[h] = dpool.tile([1, 512], f32, tag="den", name="den")
                    if os.environ.get("ATTN_RECIP", "fast") == "fast":
                        dsrc = dpool.tile([1, 512], f32, tag="dsrc", name="dsrc")
                        # tail: ScalarE is idle after the last exp -- staging
                        # the denominator there keeps the PE-idle gap under
                        # the ~3.4us HAM re-throttle window.
                        if tail:
                            nc.scalar.copy(dsrc, pv[h][D : D + 1, :])
                        else:
                            nc.vector.tensor_copy(dsrc, pv[h][D : D + 1, :])
                        nc.vector.reciprocal_approx_fast(out=dens[h], in_=dsrc)
                    else:
                        nc.vector.reciprocal(dens[h], pv[h][D : D + 1, :])
                for h in heads:
                    rbcs[h] = dpool.tile([64, 512], f32, tag="rbc", name="rbc")
                    nc.gpsimd.partition_broadcast(rbcs[h], dens[h])
                if not tail:
                    for h in heads:
                        hb = (h % 2) * 64
                        nc.vector.tensor_tensor(
                            out=outT_sb[hb : hb + 64, hp, rc * 512 : (rc + 1) * 512],
                            in0=pv[h][0:D, :],
                            in1=rbcs[h][:],
                            op=mybir.AluOpType.mult,
                        )
                else:
                    # final block: normalize per 128-token chunk and launch
                    # that chunk's out-proj + DMA immediately, so the tail
                    # pipeline (mult -> proj MM -> cast -> DMA) overlaps
                    # instead of serializing after the whole block.
                    for tc4 in range(4):
                        ts = slice(rc * 512 + tc4 * 128, rc * 512 + tc4 * 128 + 128)
                        for h in heads:
                            hb = (h % 2) * 64
                            nc.vector.tensor_tensor(
                                out=outT_sb[hb : hb + 64, hp, ts],
                                in0=pv[h][0:D, tc4 * 128 : (tc4 + 1) * 128],
                                in1=rbcs[h][:, tc4 * 128 : (tc4 + 1) * 128],
                                op=mybir.AluOpType.mult,
                            )
                        proj_tail(4 * rc + tc4)
                if hp == 1 and rc < 3:
                    queue_proj(rc)

            for sup in range(64):
                s0, s1 = 2 * sup, 2 * sup + 1
                stA = wave(s0)
                stB = wave(s1)
                act(s0, stA)
                act(s1, stB)
                if s0 >= 4:
                    pv_pair(s0 - 4)
                    pv_pair(s1 - 4)
                for _ in range(sched[sup]):
                    if fills:
                        fills.pop(0)()
            pv_pair(124)
            pv_pair(125)
            pv_pair(126)
            pv_pair(127)
            # drain any straggler fills (none expected)
            while fills:
                fills.pop(0)()
    nc.compile()
    return nc


def make_in_maps(x, w_qkv, w_proj):
    in_maps = []
    for core in range(NCORES):
        b, g = core // 4, core % 4
        qs = slice(g * 256, (g + 1) * 256)
        in_maps.append(
            {
                "xT": _prep(x[b].T),
                "wqk": _prep(
                    np.concatenate(
                        [w_qkv[:, qs], w_qkv[:, C + g * 256 : C + (g + 1) * 256]],
                        axis=1,
                    )
                ),
                "wv": _prep(w_qkv[:, 2 * C + g * 256 : 2 * C + (g + 1) * 256]),
                "wp": _prep(w_proj[qs, :]),
            }
        )
    return in_maps


def run_hw(x, w_qkv, w_proj, b_proj, trace=False, tmpdir=None):
    """Returns (full output [2, 2048, 1024] f32, exec_time_ns or None)."""
    in_maps = make_in_maps(x, w_qkv, w_proj)
    nc = build_nc()
    r = run_bass_kernel_spmd(
        nc, in_maps, core_ids=list(range(NCORES)), trace=trace, tmpdir=tmpdir
    )
    full = np.zeros((B, N, C), np.float32)
    for core in range(NCORES):
        full[core // 4] += np.asarray(r.results[core]["out"], dtype=np.float32)
    full += np.asarray(b_proj, np.float32)[None, None, :]
    return full, r.exec_time_ns


def kernel(**inputs):
    x = np.asarray(inputs["x"], np.float32)
    w_qkv = np.asarray(inputs["w_qkv"], np.float32)
    w_proj = np.asarray(inputs["w_proj"], np.float32)
    b_proj = np.asarray(inputs["b_proj"], np.float32)
    out, _ = run_hw(x, w_qkv, w_proj, b_proj, trace=False)
    return out
